# revision 24
# baseline (speedup 1.0000x reference)
"""MoE layer (E=8 experts, top-2 routing) on 8 Trainium2 NeuronCores.

Strategy: expert-parallel. The (cheap) router runs on host in fp32 numpy,
exactly reproducing the reference's softmax/top-k semantics. Tokens are
gathered per expert on host, padded to a common capacity C, and each core
runs one expert's MLP (gelu(x@W1.T+b1)@W2.T+b2, scaled by the gate) over
its token batch in fp16 matmuls with fp32 accumulation. Host scatter-adds
the two expert contributions per token back together.

Device layout notes (per core):
  matmul computes out[m,n] = sum_p lhsT[p,m]*rhs[p,n]; contraction on the
  SBUF partition dim. All operands are pre-laid-out on host so every DMA is
  contiguous:
    xt  flat [C*D], packed (block, d_inner, d_tile, token) — per-block
        partition-major so each DMA is contiguous on both sides; few,
        large transfers (dma_start triggers cost ~600ns each on the Sync
        sequencer), with the startup-critical chunks split out so the
        first matmul chains start before their block fully lands
    w1s [32,128,8,128]  W1.T blocks: (h_tile, d_inner, d_tile, h_inner)
    w2s [8,128,32,128]  W2.T blocks: (d_tile, h_inner, h_tile, d_inner)
    b1t [128,32], b2t [128,8]  biases, partition-major
    gb  [128,C]     gate weights broadcast across partitions
    yt  [8,128,C]   output y.T: (d_tile, d_inner, token)
"""

import os
import sys
import types

import numpy as np

D = 1024
H = 4096
E = 8
TOPK = 2
P = 128
NCORES = 8
DT = D // P   # 8 d-tiles
HT = H // P   # 32 h-tiles
TBLOCK = 512  # psum free-dim block


def _install_axon_hooks_shim():
    """Provide antenv.axon_hooks if the container's antenv stub lacks it.

    concourse.bass_utils imports it unconditionally when tracing under
    axon; trn_agent_boot registers the real ntff hook only when the module
    exists at boot. Installing the shim (and re-registering the hook) makes
    trace=True/BASS_TRACE=1 work instead of raising ModuleNotFoundError.
    """
    try:
        import antenv
    except ImportError:
        return
    if "antenv.axon_hooks" in sys.modules:
        return
    try:
        from antenv import axon_hooks  # noqa: F401
        return
    except ImportError:
        pass
    mod = types.ModuleType("antenv.axon_hooks")
    mod._hook = None

    def set_axon_ntff_profile_hook(h):
        mod._hook = h

    def get_axon_ntff_profile_hook():
        return mod._hook

    mod.set_axon_ntff_profile_hook = set_axon_ntff_profile_hook
    mod.get_axon_ntff_profile_hook = get_axon_ntff_profile_hook
    sys.modules["antenv.axon_hooks"] = mod
    antenv.axon_hooks = mod
    try:
        from trn_agent_boot.trn_boot import _ntff_profile_via_ctypes

        hook = _ntff_profile_via_ctypes("/opt/axon/libaxon_pjrt.so")
        if hook is not None:
            set_axon_ntff_profile_hook(hook)
    except Exception:
        pass


def _split_equal(C, nb):
    """Split C (a multiple of 16) into nb chunks, each a multiple of 16 —
    PE rhs reads at non-16-element-aligned SBUF offsets run measurably
    slower, so block boundaries must stay aligned."""
    base = (C // nb) // 16 * 16
    rem = C - base * nb
    assert rem % 16 == 0
    extra = rem // 16
    return [base + 16 * (1 if i < extra else 0) for i in range(nb)]


def _offsets(sizes):
    blocks = []
    t0 = 0
    for tb in sizes:
        blocks.append((t0, tb))
        t0 += tb
    return blocks


def _blocks_for(C):
    """Phase A blocks: big first block (512) — phase A runs block-outer, so
    during the DMA-ramp window each 512-wide chain consumes one w1 tile per
    ~1.7us (~150 GB/s), which the DMA engines can sustain from the start."""
    if C <= TBLOCK:
        return [(0, C)]
    rest = C - TBLOCK
    sizes = [TBLOCK] + _split_equal(rest, -(-rest // TBLOCK))
    return _offsets(sizes)


def _blocks_b_for(C):
    """Phase B blocks: equal chunks <=512 (no startup concern)."""
    return _offsets(_split_equal(C, -(-C // TBLOCK)))


_KERNEL_CACHE = {}


def _build_kernel(C, with_b2=True):
    """Build + compile the per-core Bass program for capacity C.

    with_b2=False (b2 all zeros, as in the reference) drops the ACT
    bias-add hop: the DVE gate-multiply reads PSUM directly."""
    import concourse.bacc as bacc
    import concourse.mybir as mybir
    import concourse.tile as tile
    from concourse.tile_rust import add_dep_helper

    dt = mybir.dt
    blocks = _blocks_for(C)
    blocks_b = _blocks_b_for(C)

    nc = bacc.Bacc("TRN2", target_bir_lowering=False, debug=False)

    # xt packed per token-block, partition-major: (bi, p, d_tile, t).
    xt = nc.dram_tensor("xt", [P * C * DT], dt.float16, kind="ExternalInput")
    w1s = nc.dram_tensor("w1s", [HT, P, DT, P], dt.float16, kind="ExternalInput")
    w2s = nc.dram_tensor("w2s", [DT, P, HT, P], dt.float16, kind="ExternalInput")
    b1t = nc.dram_tensor("b1t", [P, HT], dt.float32, kind="ExternalInput")
    b2t = (
        nc.dram_tensor("b2t", [P, DT], dt.float32, kind="ExternalInput")
        if with_b2
        else None
    )
    gb = nc.dram_tensor("gb", [P, C], dt.float32, kind="ExternalInput")
    yt = nc.dram_tensor("yt", [DT, P, C], dt.float16, kind="ExternalOutput")

    with tile.TileContext(nc) as tc:
        with (
            tc.tile_pool(name="pers", bufs=1) as pers,
            tc.tile_pool(name="w2pool", bufs=3) as w2pool,
            tc.tile_pool(name="outpool", bufs=4) as outpool,
            # One pool holding all 8 PSUM banks; every tile here is <=1 bank
            # (512 fp32), so the shared tag rotates through all 8 banks —
            # phase B keeps 3 accumulators live plus 3 draining plus slack.
            tc.tile_pool(name="psum", bufs=8, space="PSUM") as psum,
        ):
            # PE warm-up: the HAM clock gate holds the PE at 1.2 GHz until it
            # has been busy ~3.4us. Real matmuls can't start until ~9us (BSP
            # preamble + first DMA), so without this the first ~12us of real
            # work runs at half clock. A stream of dummy matmuls on a zeroed
            # scratch tile starts as soon as the engines boot (~7us) and has
            # the PE at full clock right as the first real operand lands.
            wz = pers.tile([P, P], dt.float16, tag="wz")
            nc.vector.memset(wz[:], 0.0)
            psw = psum.tile([P, P], dt.float32, name="psw", tag="ps")
            for _ in range(46):
                nc.tensor.matmul(psw[:], wz[:], wz[:], start=True, stop=True)

            # Input streams ride the two HWDGE rings in parallel: w1/w2/b/g on
            # the SP ring (nc.sync), xt + outputs on the ACT ring (nc.scalar)
            # — trigger issue serializes ~600ns apiece per ring, so splitting
            # halves time-to-first-byte at startup and keeps phase-B weight
            # loads FIFO-independent of output stores.
            #
            # All 32 w1 tiles stay resident (64 KiB/partition): the block-0
            # pass streams them in, the later block passes reuse them with no
            # DMA dependence at all.
            w1_res = [
                pers.tile([P, DT, P], dt.float16, name=f"w1r_{ht}", tag=f"w1r_{ht}")
                for ht in range(HT)
            ]
            # First weight tile arrives in d-pair slices so chain 0's first
            # LDWEIGHTS only waits on 64KB — the chain starts (cold) as soon
            # as the DMA ramp delivers the leading slices.
            for d0 in range(0, DT, 2):
                nc.sync.dma_start(w1_res[0][:, d0 : d0 + 2, :], w1s[0][:, d0 : d0 + 2, :])
            # Per-block xt tiles, packed partition-major (p, d, t): both DMA
            # sides are contiguous per partition, so transfers run at full
            # packet size even during the DMA-engine ramp. Block 0 is split
            # by d so the first chain starts as soon as d0:2 lands; blocks
            # 1+ are emitted mid-pass-0 (lower priority) so they don't steal
            # DMA bandwidth from the startup-critical block-0/w1 stream.
            xt_tiles = []
            for bi, (t0, tb) in enumerate(blocks):
                xt_tiles.append(
                    pers.tile([P, DT, tb], dt.float16, name=f"xt_{bi}", tag=f"xt_{bi}")
                )

            def _load_xt(bi, after=None):
                t0, tb = blocks[bi]
                base = P * DT * sum(b[1] for b in blocks[:bi])
                src = xt[base : base + P * DT * tb].rearrange(
                    "(p d t) -> p d t", d=DT, t=tb
                )
                d_splits = (
                    [(0, 2), (2, 4), (4, 6), (6, DT)] if bi == 0 else [(0, DT)]
                )
                for d0, d1 in d_splits:
                    di = nc.scalar.dma_start(xt_tiles[bi][:, d0:d1, :], src[:, d0:d1, :])
                    if after is not None:
                        # Static-order pin: without it the Tile scheduler
                        # floats these triggers ahead of the (still-blocked)
                        # pass-0 activations and the transfers steal DMA
                        # lanes + HBM bandwidth from the startup-critical
                        # w1/xt-block-0 stream.
                        add_dep_helper(
                            di.ins,
                            after.ins,
                            sync=False,
                            reason="hold late xt blocks behind pass-0",
                        )

            _load_xt(0)
            nc.sync.dma_start(w1_res[1][:], w1s[1])
            b1_sb = pers.tile([P, HT], dt.float32, tag="b1_sb")
            nc.sync.dma_start(b1_sb[:], b1t[:])
            h1_all = pers.tile([P, HT, C], dt.float16, tag="h1_all")

            # Phase A: h1 = gelu(x @ W1.T + b1), laid out [h_inner, h_tile, t].
            # Pass 0 (tb=512, block-outer) streams w1 at a rate the ramping
            # DMA engines can sustain; the remaining block passes reuse the
            # resident w1 tiles with no DMA dependence.
            for ht in range(HT):
                t0, tb = blocks[0]
                if ht >= 2:
                    nc.sync.dma_start(w1_res[ht][:], w1s[ht])
                ps1 = psum.tile([P, tb], dt.float32, name="ps1", tag="ps")
                for d in range(DT):
                    nc.tensor.matmul(
                        ps1[:],
                        w1_res[ht][:, d, :],
                        xt_tiles[0][:, d, :],
                        start=(d == 0),
                        stop=(d == DT - 1),
                    )
                act_i = nc.scalar.activation(
                    h1_all[:, ht, t0 : t0 + tb],
                    ps1[:],
                    mybir.ActivationFunctionType.Gelu,
                    bias=b1_sb[:, ht : ht + 1],
                )
                if ht == 5:
                    for bj in range(1, len(blocks)):
                        _load_xt(bj, after=act_i)
            for bi, (t0, tb) in enumerate(blocks[1:], start=1):
                for ht in range(HT):
                    ps1 = psum.tile([P, tb], dt.float32, name="ps1r", tag="ps")
                    for d in range(DT):
                        nc.tensor.matmul(
                            ps1[:],
                            w1_res[ht][:, d, :],
                            xt_tiles[bi][:, d, :],
                            start=(d == 0),
                            stop=(d == DT - 1),
                        )
                    nc.scalar.activation(
                        h1_all[:, ht, t0 : t0 + tb],
                        ps1[:],
                        mybir.ActivationFunctionType.Gelu,
                        bias=b1_sb[:, ht : ht + 1],
                    )

            # Phase B inputs (not needed for ~half the kernel; queue them
            # behind the w1 stream so they don't delay phase A's weights).
            if with_b2:
                b2_sb = pers.tile([P, DT], dt.float32, tag="b2_sb")
                nc.sync.dma_start(b2_sb[:], b2t[:])
            g_sb = pers.tile([P, C], dt.float32, tag="g_sb")
            nc.sync.dma_start(g_sb[:], gb[:])

            # Phase B: y = (h1 @ W2.T + b2) * gate, laid out [d_inner, t].
            # Sequential blocks per dti: earlier blocks' gate-multiply and
            # output DMA drain while later chains stream, so only the very
            # last block's epilogue trails the final matmul.
            for dti in range(DT):
                w2_t = w2pool.tile([P, HT, P], dt.float16, tag="w2_t")
                nc.sync.dma_start(w2_t[:], w2s[dti])
                for bi, (t0, tb) in enumerate(blocks_b):
                    ps2 = psum.tile([P, tb], dt.float32, name="ps2", tag="ps")
                    for ht in range(HT):
                        nc.tensor.matmul(
                            ps2[:],
                            w2_t[:, ht, :],
                            h1_all[:, ht, t0 : t0 + tb],
                            start=(ht == 0),
                            stop=(ht == HT - 1),
                        )
                    ot = outpool.tile([P, tb], dt.float16, tag="ot")
                    if with_b2:
                        nc.scalar.activation(
                            ot[:],
                            ps2[:],
                            mybir.ActivationFunctionType.Identity,
                            bias=b2_sb[:, dti : dti + 1],
                        )
                        nc.vector.tensor_mul(ot[:], ot[:], g_sb[:, t0 : t0 + tb])
                    else:
                        nc.vector.tensor_mul(ot[:], ps2[:], g_sb[:, t0 : t0 + tb])
                    nc.scalar.dma_start(yt[dti][:, t0 : t0 + tb], ot[:])

    nc.compile()
    return nc


# Results of the most recent device run (for test harness introspection).
LAST_RESULTS = None


def kernel(x, Wr, br, W1, b1, W2, b2):
    global LAST_RESULTS
    _install_axon_hooks_shim()
    from concourse.bass_utils import run_bass_kernel_spmd

    x = np.asarray(x, dtype=np.float32)
    Wr = np.asarray(Wr, dtype=np.float32)
    br = np.asarray(br, dtype=np.float32)
    W1 = np.asarray(W1, dtype=np.float32)
    b1 = np.asarray(b1, dtype=np.float32)
    W2 = np.asarray(W2, dtype=np.float32)
    b2 = np.asarray(b2, dtype=np.float32)

    B, S, Din = x.shape
    assert Din == D
    T = B * S
    x_flat = x.reshape(T, D)

    # --- Router (host, fp32, matches reference semantics) ---
    logits = x_flat @ Wr.T + br
    m = logits.max(axis=-1, keepdims=True)
    p = np.exp(logits - m)
    gates = p / p.sum(axis=-1, keepdims=True)
    # top-k, descending, ties -> lower index (matches jax.lax.top_k)
    top_i = np.argsort(-gates, axis=-1, kind="stable")[:, :TOPK]

    # --- Dispatch plan ---
    sel = np.zeros((T, E), dtype=bool)
    sel[np.arange(T)[:, None], top_i] = True
    idx_list = [np.flatnonzero(sel[:, e]) for e in range(E)]
    counts = np.array([len(ix) for ix in idx_list])
    C = max(512, int(-(-counts.max() // 16) * 16))

    pos = np.empty((T, TOPK), dtype=np.int64)
    posmap = np.empty(T, dtype=np.int64)
    for e in range(E):
        ix = idx_list[e]
        posmap[ix] = np.arange(len(ix))
        for k in range(TOPK):
            mask = top_i[:, k] == e
            pos[mask, k] = posmap[mask]

    # --- Build per-core inputs ---
    f16 = np.float16
    in_maps = []
    for e in range(E):
        ix = idx_list[e]
        n = len(ix)
        xe = np.zeros((C, D), dtype=np.float32)
        xe[:n] = x_flat[ix]
        xeT = np.ascontiguousarray(xe.T).astype(f16).reshape(DT, P, C)
        blocks = _blocks_for(C)
        xt_e = np.concatenate(
            [
                np.ascontiguousarray(xeT[:, :, t0 : t0 + tb].transpose(1, 0, 2)).reshape(-1)
                for (t0, tb) in blocks
            ]
        )
        w1s_e = np.ascontiguousarray(
            W1[e].T.reshape(DT, P, HT, P).transpose(2, 1, 0, 3)
        ).astype(f16)
        w2s_e = np.ascontiguousarray(
            W2[e].T.reshape(HT, P, DT, P).transpose(2, 1, 0, 3)
        ).astype(f16)
        b1t_e = np.ascontiguousarray(b1[e].reshape(HT, P).T)
        b2t_e = np.ascontiguousarray(b2[e].reshape(DT, P).T)
        g = np.zeros(C, dtype=np.float32)
        g[:n] = gates[ix, e]
        gb_e = np.ascontiguousarray(np.broadcast_to(g, (P, C)))
        im = {
            "xt": xt_e,
            "w1s": w1s_e,
            "w2s": w2s_e,
            "b1t": b1t_e,
            "gb": gb_e,
        }
        if bool(np.any(b2)):
            im["b2t"] = b2t_e
        in_maps.append(im)

    # --- Compile (cached) + run on 8 cores ---
    with_b2 = bool(np.any(b2))
    key = (C, with_b2)
    if key not in _KERNEL_CACHE:
        _KERNEL_CACHE[key] = _build_kernel(C, with_b2)
    nc = _KERNEL_CACHE[key]

    trace = bool(int(os.environ.get("MOE_KERNEL_TRACE", "0")))
    res = None
    last_exc = None
    for attempt in range(3):
        try:
            res = run_bass_kernel_spmd(
                nc, in_maps, core_ids=list(range(NCORES)), trace=trace
            )
            break
        except Exception as e:  # transient axon/NRT hiccups — retry
            last_exc = e
            trace = False
    if res is None:
        raise last_exc
    LAST_RESULTS = res

    # --- Combine (host): out[t] = sum_k gate_k * y_{expert_k}[t] ---
    yall = np.stack(
        [res.results[e]["yt"].reshape(D, C).T.astype(np.float32) for e in range(E)]
    )  # [E, C, D] (already gate-scaled on device; yt is fp16 to halve DMA)
    out_flat = (
        yall[top_i[:, 0], pos[:, 0]] + yall[top_i[:, 1], pos[:, 1]]
    ).astype(np.float32)
    return out_flat.reshape(B, S, Din)



# revision 25
# speedup vs baseline: 1.0183x; 1.0183x over previous
"""MoE layer (E=8 experts, top-2 routing) on 8 Trainium2 NeuronCores.

Strategy: expert-parallel. The (cheap) router runs on host in fp32 numpy,
exactly reproducing the reference's softmax/top-k semantics. Tokens are
gathered per expert on host, padded to a common capacity C, and each core
runs one expert's MLP (gelu(x@W1.T+b1)@W2.T+b2, scaled by the gate) over
its token batch in fp16 matmuls with fp32 accumulation. Host scatter-adds
the two expert contributions per token back together.

Device layout notes (per core):
  matmul computes out[m,n] = sum_p lhsT[p,m]*rhs[p,n]; contraction on the
  SBUF partition dim. All operands are pre-laid-out on host so every DMA is
  contiguous:
    xt  flat [C*D], packed (block, d_inner, d_tile, token) — per-block
        partition-major so each DMA is contiguous on both sides; few,
        large transfers (dma_start triggers cost ~600ns each on the Sync
        sequencer), with the startup-critical chunks split out so the
        first matmul chains start before their block fully lands
    w1s [32,128,8,128]  W1.T blocks: (h_tile, d_inner, d_tile, h_inner)
    w2s [8,128,32,128]  W2.T blocks: (d_tile, h_inner, h_tile, d_inner)
    b1t [128,32], b2t [128,8]  biases, partition-major
    gb  [128,C]     gate weights broadcast across partitions
    yt  [8,128,C]   output y.T: (d_tile, d_inner, token)
"""

import os
import sys
import types

import numpy as np

D = 1024
H = 4096
E = 8
TOPK = 2
P = 128
NCORES = 8
DT = D // P   # 8 d-tiles
HT = H // P   # 32 h-tiles
TBLOCK = 512  # psum free-dim block


def _install_axon_hooks_shim():
    """Provide antenv.axon_hooks if the container's antenv stub lacks it.

    concourse.bass_utils imports it unconditionally when tracing under
    axon; trn_agent_boot registers the real ntff hook only when the module
    exists at boot. Installing the shim (and re-registering the hook) makes
    trace=True/BASS_TRACE=1 work instead of raising ModuleNotFoundError.
    """
    try:
        import antenv
    except ImportError:
        return
    if "antenv.axon_hooks" in sys.modules:
        return
    try:
        from antenv import axon_hooks  # noqa: F401
        return
    except ImportError:
        pass
    mod = types.ModuleType("antenv.axon_hooks")
    mod._hook = None

    def set_axon_ntff_profile_hook(h):
        mod._hook = h

    def get_axon_ntff_profile_hook():
        return mod._hook

    mod.set_axon_ntff_profile_hook = set_axon_ntff_profile_hook
    mod.get_axon_ntff_profile_hook = get_axon_ntff_profile_hook
    sys.modules["antenv.axon_hooks"] = mod
    antenv.axon_hooks = mod
    try:
        from trn_agent_boot.trn_boot import _ntff_profile_via_ctypes

        hook = _ntff_profile_via_ctypes("/opt/axon/libaxon_pjrt.so")
        if hook is not None:
            set_axon_ntff_profile_hook(hook)
    except Exception:
        pass


def _split_equal(C, nb):
    """Split C (a multiple of 16) into nb chunks, each a multiple of 16 —
    PE rhs reads at non-16-element-aligned SBUF offsets run measurably
    slower, so block boundaries must stay aligned."""
    base = (C // nb) // 16 * 16
    rem = C - base * nb
    assert rem % 16 == 0
    extra = rem // 16
    return [base + 16 * (1 if i < extra else 0) for i in range(nb)]


def _offsets(sizes):
    blocks = []
    t0 = 0
    for tb in sizes:
        blocks.append((t0, tb))
        t0 += tb
    return blocks


def _blocks_for(C):
    """Phase A blocks: big first block (512) — phase A runs block-outer, so
    during the DMA-ramp window each 512-wide chain consumes one w1 tile per
    ~1.7us (~150 GB/s), which the DMA engines can sustain from the start."""
    if C <= TBLOCK:
        return [(0, C)]
    rest = C - TBLOCK
    sizes = [TBLOCK] + _split_equal(rest, -(-rest // TBLOCK))
    return _offsets(sizes)


def _blocks_b_for(C):
    """Phase B blocks: equal chunks <=512 (no startup concern)."""
    return _offsets(_split_equal(C, -(-C // TBLOCK)))


_KERNEL_CACHE = {}


def _build_kernel(C, with_b2=True):
    """Build + compile the per-core Bass program for capacity C.

    with_b2=False (b2 all zeros, as in the reference) drops the ACT
    bias-add hop: the DVE gate-multiply reads PSUM directly."""
    import concourse.bacc as bacc
    import concourse.mybir as mybir
    import concourse.tile as tile
    from concourse.tile_rust import add_dep_helper

    dt = mybir.dt
    blocks = _blocks_for(C)
    blocks_b = _blocks_b_for(C)

    nc = bacc.Bacc("TRN2", target_bir_lowering=False, debug=False)

    # xt packed per token-block, partition-major: (bi, p, d_tile, t).
    xt = nc.dram_tensor("xt", [P * C * DT], dt.float16, kind="ExternalInput")
    w1s = nc.dram_tensor("w1s", [HT, P, DT, P], dt.float16, kind="ExternalInput")
    w2s = nc.dram_tensor("w2s", [DT, P, HT, P], dt.float16, kind="ExternalInput")
    b1t = nc.dram_tensor("b1t", [P, HT], dt.float32, kind="ExternalInput")
    b2t = (
        nc.dram_tensor("b2t", [P, DT], dt.float32, kind="ExternalInput")
        if with_b2
        else None
    )
    gb = nc.dram_tensor("gb", [P, C], dt.float32, kind="ExternalInput")
    yt = nc.dram_tensor("yt", [DT, P, C], dt.float16, kind="ExternalOutput")

    with tile.TileContext(nc) as tc:
        with (
            tc.tile_pool(name="pers", bufs=1) as pers,
            tc.tile_pool(name="w2pool", bufs=3) as w2pool,
            tc.tile_pool(name="outpool", bufs=4) as outpool,
            # One pool holding all 8 PSUM banks; every tile here is <=1 bank
            # (512 fp32), so the shared tag rotates through all 8 banks —
            # phase B keeps 3 accumulators live plus 3 draining plus slack.
            tc.tile_pool(name="psum", bufs=8, space="PSUM") as psum,
        ):
            # PE warm-up: the HAM clock gate holds the PE at 1.2 GHz until it
            # has been busy ~3.4us. Real matmuls can't start until ~9us (BSP
            # preamble + first DMA), so without this the first ~12us of real
            # work runs at half clock. A stream of dummy matmuls on a zeroed
            # scratch tile starts as soon as the engines boot (~7us) and has
            # the PE at full clock right as the first real operand lands.
            wz = pers.tile([P, P], dt.float16, tag="wz")
            nc.vector.memset(wz[:], 0.0)
            # ~29 dummies run cold (112ns) before the HAM un-throttles; the
            # rest run warm (~50ns). 80 keeps the PE continuously busy until
            # ~14us — past the worst-case arrival of chain 0's operands — so
            # the activity window never breaks and real work starts at full
            # clock. A too-short bridge is expensive: any early >1us stall
            # re-throttles the PE to 1.2 GHz for ~3.4us of real work.
            psw = psum.tile([P, P], dt.float32, name="psw", tag="ps")
            for _ in range(80):
                nc.tensor.matmul(psw[:], wz[:], wz[:], start=True, stop=True)

            # Input streams ride the two HWDGE rings in parallel: w1/w2/b/g on
            # the SP ring (nc.sync), xt + outputs on the ACT ring (nc.scalar)
            # — trigger issue serializes ~600ns apiece per ring, so splitting
            # halves time-to-first-byte at startup and keeps phase-B weight
            # loads FIFO-independent of output stores.
            #
            # All 32 w1 tiles stay resident (64 KiB/partition): the block-0
            # pass streams them in, the later block passes reuse them with no
            # DMA dependence at all.
            w1_res = [
                pers.tile([P, DT, P], dt.float16, name=f"w1r_{ht}", tag=f"w1r_{ht}")
                for ht in range(HT)
            ]
            # First weight tile arrives in d-pair slices so chain 0's first
            # LDWEIGHTS only waits on 64KB — the chain starts (cold) as soon
            # as the DMA ramp delivers the leading slices.
            for d0 in range(0, DT, 2):
                nc.sync.dma_start(w1_res[0][:, d0 : d0 + 2, :], w1s[0][:, d0 : d0 + 2, :])
            # Per-block xt tiles, packed partition-major (p, d, t): both DMA
            # sides are contiguous per partition, so transfers run at full
            # packet size even during the DMA-engine ramp. Block 0 is split
            # by d so the first chain starts as soon as d0:2 lands; blocks
            # 1+ are emitted mid-pass-0 (lower priority) so they don't steal
            # DMA bandwidth from the startup-critical block-0/w1 stream.
            xt_tiles = []
            for bi, (t0, tb) in enumerate(blocks):
                xt_tiles.append(
                    pers.tile([P, DT, tb], dt.float16, name=f"xt_{bi}", tag=f"xt_{bi}")
                )

            def _load_xt(bi, after=None):
                t0, tb = blocks[bi]
                base = P * DT * sum(b[1] for b in blocks[:bi])
                src = xt[base : base + P * DT * tb].rearrange(
                    "(p d t) -> p d t", d=DT, t=tb
                )
                d_splits = (
                    [(0, 2), (2, 4), (4, 6), (6, DT)] if bi == 0 else [(0, DT)]
                )
                for d0, d1 in d_splits:
                    di = nc.scalar.dma_start(xt_tiles[bi][:, d0:d1, :], src[:, d0:d1, :])
                    if after is not None:
                        # Static-order pin: without it the Tile scheduler
                        # floats these triggers ahead of the (still-blocked)
                        # pass-0 activations and the transfers steal DMA
                        # lanes + HBM bandwidth from the startup-critical
                        # w1/xt-block-0 stream.
                        add_dep_helper(
                            di.ins,
                            after.ins,
                            sync=False,
                            reason="hold late xt blocks behind pass-0",
                        )

            _load_xt(0)
            nc.sync.dma_start(w1_res[1][:], w1s[1])
            b1_sb = pers.tile([P, HT], dt.float32, tag="b1_sb")
            nc.sync.dma_start(b1_sb[:], b1t[:])
            h1_all = pers.tile([P, HT, C], dt.float16, tag="h1_all")

            # Phase A: h1 = gelu(x @ W1.T + b1), laid out [h_inner, h_tile, t].
            # Pass 0 (tb=512, block-outer) streams w1 at a rate the ramping
            # DMA engines can sustain; the remaining block passes reuse the
            # resident w1 tiles with no DMA dependence.
            for ht in range(HT):
                t0, tb = blocks[0]
                if ht >= 2:
                    nc.sync.dma_start(w1_res[ht][:], w1s[ht])
                ps1 = psum.tile([P, tb], dt.float32, name="ps1", tag="ps")
                for d in range(DT):
                    nc.tensor.matmul(
                        ps1[:],
                        w1_res[ht][:, d, :],
                        xt_tiles[0][:, d, :],
                        start=(d == 0),
                        stop=(d == DT - 1),
                    )
                act_i = nc.scalar.activation(
                    h1_all[:, ht, t0 : t0 + tb],
                    ps1[:],
                    mybir.ActivationFunctionType.Gelu,
                    bias=b1_sb[:, ht : ht + 1],
                )
                if ht == 5:
                    for bj in range(1, len(blocks)):
                        _load_xt(bj, after=act_i)
            for bi, (t0, tb) in enumerate(blocks[1:], start=1):
                for ht in range(HT):
                    ps1 = psum.tile([P, tb], dt.float32, name="ps1r", tag="ps")
                    for d in range(DT):
                        nc.tensor.matmul(
                            ps1[:],
                            w1_res[ht][:, d, :],
                            xt_tiles[bi][:, d, :],
                            start=(d == 0),
                            stop=(d == DT - 1),
                        )
                    nc.scalar.activation(
                        h1_all[:, ht, t0 : t0 + tb],
                        ps1[:],
                        mybir.ActivationFunctionType.Gelu,
                        bias=b1_sb[:, ht : ht + 1],
                    )

            # Phase B inputs (not needed for ~half the kernel; queue them
            # behind the w1 stream so they don't delay phase A's weights).
            if with_b2:
                b2_sb = pers.tile([P, DT], dt.float32, tag="b2_sb")
                nc.sync.dma_start(b2_sb[:], b2t[:])
            g_sb = pers.tile([P, C], dt.float32, tag="g_sb")
            nc.sync.dma_start(g_sb[:], gb[:])

            # Phase B: y = (h1 @ W2.T + b2) * gate, laid out [d_inner, t].
            # Sequential blocks per dti: earlier blocks' gate-multiply and
            # output DMA drain while later chains stream, so only the very
            # last block's epilogue trails the final matmul.
            for dti in range(DT):
                w2_t = w2pool.tile([P, HT, P], dt.float16, tag="w2_t")
                nc.sync.dma_start(w2_t[:], w2s[dti])
                for bi, (t0, tb) in enumerate(blocks_b):
                    ps2 = psum.tile([P, tb], dt.float32, name="ps2", tag="ps")
                    for ht in range(HT):
                        nc.tensor.matmul(
                            ps2[:],
                            w2_t[:, ht, :],
                            h1_all[:, ht, t0 : t0 + tb],
                            start=(ht == 0),
                            stop=(ht == HT - 1),
                        )
                    ot = outpool.tile([P, tb], dt.float16, tag="ot")
                    if with_b2:
                        nc.scalar.activation(
                            ot[:],
                            ps2[:],
                            mybir.ActivationFunctionType.Identity,
                            bias=b2_sb[:, dti : dti + 1],
                        )
                        nc.vector.tensor_mul(ot[:], ot[:], g_sb[:, t0 : t0 + tb])
                    else:
                        nc.vector.tensor_mul(ot[:], ps2[:], g_sb[:, t0 : t0 + tb])
                    nc.scalar.dma_start(yt[dti][:, t0 : t0 + tb], ot[:])

    nc.compile()
    return nc


# Results of the most recent device run (for test harness introspection).
LAST_RESULTS = None


def kernel(x, Wr, br, W1, b1, W2, b2):
    global LAST_RESULTS
    _install_axon_hooks_shim()
    from concourse.bass_utils import run_bass_kernel_spmd

    x = np.asarray(x, dtype=np.float32)
    Wr = np.asarray(Wr, dtype=np.float32)
    br = np.asarray(br, dtype=np.float32)
    W1 = np.asarray(W1, dtype=np.float32)
    b1 = np.asarray(b1, dtype=np.float32)
    W2 = np.asarray(W2, dtype=np.float32)
    b2 = np.asarray(b2, dtype=np.float32)

    B, S, Din = x.shape
    assert Din == D
    T = B * S
    x_flat = x.reshape(T, D)

    # --- Router (host, fp32, matches reference semantics) ---
    logits = x_flat @ Wr.T + br
    m = logits.max(axis=-1, keepdims=True)
    p = np.exp(logits - m)
    gates = p / p.sum(axis=-1, keepdims=True)
    # top-k, descending, ties -> lower index (matches jax.lax.top_k)
    top_i = np.argsort(-gates, axis=-1, kind="stable")[:, :TOPK]

    # --- Dispatch plan ---
    sel = np.zeros((T, E), dtype=bool)
    sel[np.arange(T)[:, None], top_i] = True
    idx_list = [np.flatnonzero(sel[:, e]) for e in range(E)]
    counts = np.array([len(ix) for ix in idx_list])
    C = max(512, int(-(-counts.max() // 16) * 16))

    pos = np.empty((T, TOPK), dtype=np.int64)
    posmap = np.empty(T, dtype=np.int64)
    for e in range(E):
        ix = idx_list[e]
        posmap[ix] = np.arange(len(ix))
        for k in range(TOPK):
            mask = top_i[:, k] == e
            pos[mask, k] = posmap[mask]

    # --- Build per-core inputs ---
    f16 = np.float16
    in_maps = []
    for e in range(E):
        ix = idx_list[e]
        n = len(ix)
        xe = np.zeros((C, D), dtype=np.float32)
        xe[:n] = x_flat[ix]
        xeT = np.ascontiguousarray(xe.T).astype(f16).reshape(DT, P, C)
        blocks = _blocks_for(C)
        xt_e = np.concatenate(
            [
                np.ascontiguousarray(xeT[:, :, t0 : t0 + tb].transpose(1, 0, 2)).reshape(-1)
                for (t0, tb) in blocks
            ]
        )
        w1s_e = np.ascontiguousarray(
            W1[e].T.reshape(DT, P, HT, P).transpose(2, 1, 0, 3)
        ).astype(f16)
        w2s_e = np.ascontiguousarray(
            W2[e].T.reshape(HT, P, DT, P).transpose(2, 1, 0, 3)
        ).astype(f16)
        b1t_e = np.ascontiguousarray(b1[e].reshape(HT, P).T)
        b2t_e = np.ascontiguousarray(b2[e].reshape(DT, P).T)
        g = np.zeros(C, dtype=np.float32)
        g[:n] = gates[ix, e]
        gb_e = np.ascontiguousarray(np.broadcast_to(g, (P, C)))
        im = {
            "xt": xt_e,
            "w1s": w1s_e,
            "w2s": w2s_e,
            "b1t": b1t_e,
            "gb": gb_e,
        }
        if bool(np.any(b2)):
            im["b2t"] = b2t_e
        in_maps.append(im)

    # --- Compile (cached) + run on 8 cores ---
    with_b2 = bool(np.any(b2))
    key = (C, with_b2)
    if key not in _KERNEL_CACHE:
        _KERNEL_CACHE[key] = _build_kernel(C, with_b2)
    nc = _KERNEL_CACHE[key]

    trace = bool(int(os.environ.get("MOE_KERNEL_TRACE", "0")))
    res = None
    last_exc = None
    for attempt in range(3):
        try:
            res = run_bass_kernel_spmd(
                nc, in_maps, core_ids=list(range(NCORES)), trace=trace
            )
            break
        except Exception as e:  # transient axon/NRT hiccups — retry
            last_exc = e
            trace = False
    if res is None:
        raise last_exc
    LAST_RESULTS = res

    # --- Combine (host): out[t] = sum_k gate_k * y_{expert_k}[t] ---
    yall = np.stack(
        [res.results[e]["yt"].reshape(D, C).T.astype(np.float32) for e in range(E)]
    )  # [E, C, D] (already gate-scaled on device; yt is fp16 to halve DMA)
    out_flat = (
        yall[top_i[:, 0], pos[:, 0]] + yall[top_i[:, 1], pos[:, 1]]
    ).astype(np.float32)
    return out_flat.reshape(B, S, Din)

